# revision 12
# baseline (speedup 1.0000x reference)
"""Self-contained 8-core Trainium2 Bass kernel for the BaseGNN problem.

kernel(**inputs) -> np.ndarray [50000, 72] float32.

Strategy (v2): degree-sorted node sharding across 8 NeuronCores. Per conv
layer, h' = h*dinv is transposed to a node-major bf16 table (pair-packed
[25088, 256] rows) and allgathered to all cores. Edge messages are fetched
with large batched GPSIMD dma_gather calls (int16 pair indices, even/odd
half-row parity split) — one pair of calls per group of ~4 destination
blocks — and aggregated on the TensorEngine into PSUM (lhsT=gathered chunk,
rhs=identity, accumulating feature-major). BN stats go through a tiny
AllReduce; encoder/classifier matmuls are fused in.
"""
import time

import numpy as np
import ml_dtypes

import jax
from jax.sharding import Mesh, PartitionSpec
from jax.experimental.shard_map import shard_map

import concourse.bacc as bacc
import concourse.tile as tile
import concourse.mybir as mybir
from concourse import bass
from concourse.bass2jax import _bass_exec_p, install_neuronx_cc_hook, partition_id_tensor

N = 50000
E = 1000000
bf16_np = ml_dtypes.bfloat16

F_IN = 16
HID = 128
N_CLS = 72
EPS = 1e-5
NC = 8
PER_CORE = 6250
BLOCKS = 49
LSHARD = BLOCKS * 128  # 6272
TOT = NC * LSHARD      # 50176
NPAIR = TOT // 2       # 25088
N_REAL = 50000
CAPG = 110   # max chunks per gather group
NBMAX = 4    # max blocks per group (PSUM tile is 512 cols)

f32 = mybir.dt.float32
bf16 = mybir.dt.bfloat16
i16 = mybir.dt.int16


def col_chunks(width=512):
    s = 0
    while s < LSHARD:
        w = min(width, LSHARD - s)
        yield s, w
        s += w


def build(groups, CH):
    """groups: list of dicts with keys b0, nb, gstart, ge, go,
    eloc (per-block local even-chunk base), dbe, oloc, dbo."""
    GW = max(g["ge"] + g["go"] for g in groups)
    nc = bacc.Bacc("TRN2", target_bir_lowering=False, debug=False,
                   enable_asserts=False, num_devices=NC, num_swdge_queues=1)

    # ---- inputs ----
    xT_d = nc.dram_tensor("xT", [F_IN, LSHARD], f32, kind="ExternalInput")
    idx_d = nc.dram_tensor("idx16", [128, CH * 8], i16, kind="ExternalInput")
    dinv_d = nc.dram_tensor("dinvbc", [128, LSHARD], f32, kind="ExternalInput")
    wenc_d = nc.dram_tensor("W_enc", [F_IN, HID], f32, kind="ExternalInput")
    benc_d = nc.dram_tensor("b_enc", [HID, 1], f32, kind="ExternalInput")
    wc_d = nc.dram_tensor("W_conv", [3, HID, HID], bf16, kind="ExternalInput")
    bng_d = nc.dram_tensor("bn_g", [3, HID, 1], f32, kind="ExternalInput")
    bnb_d = nc.dram_tensor("bn_b", [3, HID, 1], f32, kind="ExternalInput")
    wc1_d = nc.dram_tensor("W_cls1", [HID, 64], bf16, kind="ExternalInput")
    bc1_d = nc.dram_tensor("b_cls1", [64, 1], f32, kind="ExternalInput")
    wc2_d = nc.dram_tensor("W_cls2", [64, N_CLS], bf16, kind="ExternalInput")
    bc2_d = nc.dram_tensor("b_cls2", [N_CLS, 1], f32, kind="ExternalInput")
    ident_d = nc.dram_tensor("ident", [128, 128], bf16, kind="ExternalInput")
    out_d = nc.dram_tensor("outT", [N_CLS, LSHARD], f32, kind="ExternalOutput")

    rg = [list(range(NC))]
    NG = len(groups)

    with tile.TileContext(nc) as tc:
        with tc.tile_pool(name="persist", bufs=1) as pp, \
             tc.tile_pool(name="work", bufs=4) as wp, \
             tc.tile_pool(name="psum", bufs=2, space="PSUM") as psp, \
             tc.tile_pool(name="dram", bufs=1, space="DRAM") as dp:

            # ---- persistent SBUF ----
            hT = pp.tile([128, LSHARD], f32, name="hT")
            hpTb = pp.tile([128, LSHARD], bf16, name="hpTb")
            convT = pp.tile([128, LSHARD], f32, name="convT")
            dinv = pp.tile([128, LSHARD], f32, name="dinv")
            bounce_sb = pp.tile([128, LSHARD], bf16, name="bounce_sb")
            idx16 = pp.tile([128, CH * 8], i16, name="idx16")
            identb = pp.tile([128, 128], bf16, name="identb")
            wenc = pp.tile([F_IN, HID], f32, name="wenc")
            benc = pp.tile([HID, 1], f32, name="benc")
            wc = [pp.tile([HID, HID], bf16, name=f"wc{i}") for i in range(3)]
            bng = pp.tile([HID, 3], f32, name="bng")
            bnb = pp.tile([HID, 3], f32, name="bnb")
            wc1 = pp.tile([HID, 64], bf16, name="wc1")
            bc1 = pp.tile([64, 1], f32, name="bc1")
            wc2 = pp.tile([64, N_CLS], bf16, name="wc2")
            bc2 = pp.tile([N_CLS, 1], f32, name="bc2")
            bnst = pp.tile([128, NG * 6], f32, name="bnst")

            nc.sync.dma_start(out=dinv[:], in_=dinv_d.ap())
            nc.sync.dma_start(out=idx16[:], in_=idx_d.ap())
            nc.sync.dma_start(out=identb[:], in_=ident_d.ap())
            nc.sync.dma_start(out=wenc[:], in_=wenc_d.ap())
            nc.sync.dma_start(out=benc[:], in_=benc_d.ap())
            for l in range(3):
                nc.sync.dma_start(out=wc[l][:], in_=wc_d.ap()[l])
                nc.sync.dma_start(out=bng[:, l:l + 1], in_=bng_d.ap()[l])
                nc.sync.dma_start(out=bnb[:, l:l + 1], in_=bnb_d.ap()[l])
            nc.sync.dma_start(out=wc1[:], in_=wc1_d.ap())
            nc.sync.dma_start(out=bc1[:], in_=bc1_d.ap())
            nc.sync.dma_start(out=wc2[:], in_=wc2_d.ap())
            nc.sync.dma_start(out=bc2[:], in_=bc2_d.ap())

            # ---- encoder: hT = relu(Wenc^T @ xT + b), x streamed ----
            for s, w in col_chunks():
                xt = wp.tile([F_IN, 512], f32, tag="xt", bufs=2, name="xt")
                nc.sync.dma_start(out=xt[:, :w], in_=xT_d.ap()[:, s:s + w])
                pse = psp.tile([128, 512], f32, tag="mm", name="pse")
                nc.tensor.matmul(out=pse[:, :w], lhsT=wenc[:], rhs=xt[:, :w],
                                 start=True, stop=True)
                nc.scalar.activation(hT[:, s:s + w], pse[:, :w],
                                     mybir.ActivationFunctionType.Relu,
                                     bias=benc[:, 0:1], scale=1.0)

            # ---- conv layers ----
            for l in range(3):
                # h' = hT * dinv -> bf16
                for s, w in col_chunks():
                    nc.vector.tensor_tensor(out=hpTb[:, s:s + w], in0=hT[:, s:s + w],
                                            in1=dinv[:, s:s + w],
                                            op=mybir.AluOpType.mult)
                # transpose all blocks into bounce_sb (node-major), 4 per PSUM tile
                for t in range((BLOCKS + 3) // 4):
                    b0 = t * 4
                    nb = min(4, BLOCKS - b0)
                    pst = psp.tile([128, 512], f32, tag="mm", name="pst")
                    for k in range(nb):
                        bs = (b0 + k) * 128
                        nc.tensor.matmul(out=pst[:, k * 128:(k + 1) * 128],
                                         lhsT=hpTb[:, bs:bs + 128], rhs=identb[:],
                                         start=True, stop=True)
                    nc.vector.tensor_copy(out=bounce_sb[:, b0 * 128:(b0 + nb) * 128],
                                          in_=pst[:, :nb * 128])
                bounce = dp.tile([128, LSHARD], bf16, name=f"bounce{l}")
                table = dp.tile([NPAIR, 256], bf16, addr_space="Shared",
                                name=f"table{l}")
                nc.sync.dma_start(out=bounce[:], in_=bounce_sb[:])
                nc.gpsimd.collective_compute(
                    "AllGather", mybir.AluOpType.bypass, replica_groups=rg,
                    ins=[bounce.opt()], outs=[table.opt()])

                # gather groups + aggregation
                for gi, g in enumerate(groups):
                    ge, go, gstart = g["ge"], g["go"], g["gstart"]
                    nb, b0 = g["nb"], g["b0"]
                    gb = wp.tile([128, GW * 128], bf16, tag="gb", bufs=2,
                                 name=f"gb{l}_{gi}")
                    if ge:
                        nc.gpsimd.dma_gather(
                            out_ap=gb[:, :ge * 128].rearrange("p (c e) -> p c e", e=128),
                            in_ap=table[:, 0:128],
                            idxs_ap=idx16[:, gstart * 8:(gstart + ge) * 8],
                            num_idxs=ge * 128, num_idxs_reg=ge * 128,
                            elem_size=128, elem_step=256, single_packet=False,
                            queue_num=0)
                    if go:
                        nc.gpsimd.dma_gather(
                            out_ap=gb[:, ge * 128:(ge + go) * 128].rearrange(
                                "p (c e) -> p c e", e=128),
                            in_ap=table[:, 128:256],
                            idxs_ap=idx16[:, (gstart + ge) * 8:(gstart + ge + go) * 8],
                            num_idxs=go * 128, num_idxs_reg=go * 128,
                            elem_size=128, elem_step=256, single_packet=False,
                            queue_num=0)
                    # aggregate: psa[:, k*128:(k+1)*128] += chunk^T for block's chunks
                    psa = psp.tile([128, 512], f32, tag="agg", name="psa")
                    for k in range(nb):
                        lcs = ([g["eloc"][k] + i for i in range(g["dbe"][k])]
                               + [g["oloc"][k] + i for i in range(g["dbo"][k])])
                        for i, lc in enumerate(lcs):
                            nc.tensor.matmul(
                                out=psa[:, k * 128:(k + 1) * 128],
                                lhsT=gb[:, lc * 128:(lc + 1) * 128],
                                rhs=identb[:],
                                start=(i == 0), stop=(i == len(lcs) - 1))
                    cs = b0 * 128
                    cw = nb * 128
                    at = wp.tile([128, 512], bf16, tag="at", bufs=2, name="at")
                    nc.vector.tensor_tensor(out=at[:, :cw], in0=psa[:, :cw],
                                            in1=hpTb[:, cs:cs + cw],
                                            op=mybir.AluOpType.add)
                    psc = psp.tile([128, 512], f32, tag="mm", name="psc")
                    nc.tensor.matmul(out=psc[:, :cw], lhsT=wc[l][:], rhs=at[:, :cw],
                                     start=True, stop=True)
                    nc.vector.tensor_tensor(out=convT[:, cs:cs + cw],
                                            in0=psc[:, :cw],
                                            in1=dinv[:, cs:cs + cw],
                                            op=mybir.AluOpType.mult)
                    nc.vector.bn_stats(out=bnst[:, gi * 6:(gi + 1) * 6],
                                       in_=convT[:, cs:cs + cw])

                # global BN stats
                bnagg = wp.tile([128, 2], f32, tag="st", name="bnagg")
                nc.vector.bn_aggr(out=bnagg[:], in_=bnst[:])
                ssum = wp.tile([128, 2], f32, tag="st", name="ssum")
                # ssum[:,0] = LSHARD*mean ; ssum[:,1] = LSHARD*(var+mean^2)
                m2 = wp.tile([128, 1], f32, tag="st1", name="m2")
                nc.vector.tensor_tensor(out=m2[:], in0=bnagg[:, 0:1],
                                        in1=bnagg[:, 0:1], op=mybir.AluOpType.mult)
                nc.vector.tensor_scalar_mul(ssum[:, 0:1], bnagg[:, 0:1],
                                            float(LSHARD))
                q = wp.tile([128, 1], f32, tag="st1", name="q")
                nc.vector.tensor_tensor(out=q[:], in0=bnagg[:, 1:2], in1=m2[:],
                                        op=mybir.AluOpType.add)
                nc.vector.tensor_scalar_mul(ssum[:, 1:2], q[:], float(LSHARD))
                stat_src = dp.tile([128, 2], f32, name=f"stat_src{l}")
                stat_dst = dp.tile([128, 2], f32, addr_space="Shared",
                                   name=f"stat_dst{l}")
                nc.sync.dma_start(out=stat_src[:], in_=ssum[:])
                nc.gpsimd.collective_compute(
                    "AllReduce", mybir.AluOpType.add, replica_groups=rg,
                    ins=[stat_src.opt()], outs=[stat_dst.opt()])
                gstat = wp.tile([128, 2], f32, tag="st", name="gstat")
                nc.sync.dma_start(out=gstat[:], in_=stat_dst[:])
                mu = wp.tile([128, 1], f32, tag="st1", name="mu")
                nc.vector.tensor_scalar_mul(mu[:], gstat[:, 0:1], 1.0 / N_REAL)
                var = wp.tile([128, 1], f32, tag="st1", name="var")
                nc.vector.tensor_scalar_mul(var[:], gstat[:, 1:2], 1.0 / N_REAL)
                mu2 = wp.tile([128, 1], f32, tag="st1", name="mu2")
                nc.vector.tensor_tensor(out=mu2[:], in0=mu[:], in1=mu[:],
                                        op=mybir.AluOpType.mult)
                nc.vector.tensor_tensor(out=var[:], in0=var[:], in1=mu2[:],
                                        op=mybir.AluOpType.subtract)
                nc.vector.tensor_scalar_add(var[:], var[:], EPS)
                rinv = wp.tile([128, 1], f32, tag="st1", name="rinv")
                nc.vector.reciprocal(rinv[:], var[:])
                rs = wp.tile([128, 1], f32, tag="st1", name="rs")
                nc.scalar.sqrt(rs[:], rinv[:])
                gp = wp.tile([128, 1], f32, tag="st1", name="gp")
                nc.vector.tensor_tensor(out=gp[:], in0=bng[:, l:l + 1], in1=rs[:],
                                        op=mybir.AluOpType.mult)
                mgp = wp.tile([128, 1], f32, tag="st1", name="mgp")
                nc.vector.tensor_tensor(out=mgp[:], in0=mu[:], in1=gp[:],
                                        op=mybir.AluOpType.mult)
                bp = wp.tile([128, 1], f32, tag="st1", name="bp")
                nc.vector.tensor_tensor(out=bp[:], in0=bnb[:, l:l + 1], in1=mgp[:],
                                        op=mybir.AluOpType.subtract)

                # bn apply + relu (+ residual)
                for s, w in col_chunks():
                    if l == 0:
                        nc.scalar.activation(hT[:, s:s + w], convT[:, s:s + w],
                                             mybir.ActivationFunctionType.Relu,
                                             bias=bp[:, 0:1], scale=gp[:, 0:1])
                    else:
                        hnew = wp.tile([128, 512], f32, tag="hnew", bufs=2, name="hnew")
                        nc.scalar.activation(hnew[:, :w], convT[:, s:s + w],
                                             mybir.ActivationFunctionType.Relu,
                                             bias=bp[:, 0:1], scale=gp[:, 0:1])
                        nc.vector.tensor_tensor(out=hT[:, s:s + w],
                                                in0=hT[:, s:s + w],
                                                in1=hnew[:, :w],
                                                op=mybir.AluOpType.add)

            # ---- classifier ----
            h4b = pp.tile([64, LSHARD], bf16, name="h4b")
            for s, w in col_chunks():
                nc.vector.tensor_copy(out=hpTb[:, s:s + w], in_=hT[:, s:s + w])
                ps1 = psp.tile([64, 512], f32, tag="mmc", name="ps1")
                nc.tensor.matmul(out=ps1[:, :w], lhsT=wc1[:], rhs=hpTb[:, s:s + w],
                                 start=True, stop=True)
                nc.scalar.activation(h4b[:, s:s + w], ps1[:, :w],
                                     mybir.ActivationFunctionType.Relu,
                                     bias=bc1[:, 0:1], scale=1.0)
            for s, w in col_chunks():
                ps2 = psp.tile([N_CLS, 512], f32, tag="mmc", name="ps2")
                nc.tensor.matmul(out=ps2[:, :w], lhsT=wc2[:], rhs=h4b[:, s:s + w],
                                 start=True, stop=True)
                ot = wp.tile([N_CLS, 512], f32, tag="ot", bufs=2, name="ot")
                nc.vector.tensor_scalar(out=ot[:, :w], in0=ps2[:, :w],
                                        scalar1=bc2[:, 0:1], scalar2=None,
                                        op0=mybir.AluOpType.add)
                nc.sync.dma_start(out=out_d.ap()[:, s:s + w], in_=ot[:, :w])

    nc.compile()
    return nc


# ---------------- host preprocessing ----------------
def preprocess(edge_index):
    src = np.asarray(edge_index[0], dtype=np.int64)
    dst = np.asarray(edge_index[1], dtype=np.int64)
    indeg = np.bincount(dst, minlength=N).astype(np.int64)
    deg = (indeg + 1).astype(np.float32)
    dinv = (1.0 / np.sqrt(deg)).astype(np.float32)

    # snake-deal nodes sorted by indeg desc -> 8 cores x 6250, edge-balanced
    order = np.argsort(-indeg, kind="stable")
    rounds = PER_CORE // 2
    po = order.reshape(rounds, 2, NC)
    core_nodes = np.empty((NC, PER_CORE), dtype=np.int64)
    core_nodes[:, 0::2] = po[:, 0, :].T
    core_nodes[:, 1::2] = po[:, 1, ::-1].T
    for c in range(NC):
        o = np.argsort(-indeg[core_nodes[c]], kind="stable")
        core_nodes[c] = core_nodes[c][o]

    flat = core_nodes.ravel()
    node_c = np.empty(N, np.int64)
    node_j = np.empty(N, np.int64)
    node_c[flat] = np.repeat(np.arange(NC), PER_CORE)
    node_j[flat] = np.tile(np.arange(PER_CORE), NC)
    b_of = node_j // 128

    # ---- parity balancing ----
    # slot parity of position p in block b is (p+b)%2: 64 even + 64 odd slots
    # per block. Greedily pick each source's parity to balance its dsts'
    # even/odd neighbor counts (bounds the per-block chunk count Dbe+Dbo).
    e_by_src = np.argsort(src, kind="stable")
    d_sorted = dst[e_by_src]
    sstarts = np.searchsorted(src[e_by_src], np.arange(N + 1))
    outdeg = sstarts[1:] - sstarts[:-1]
    proc = np.argsort(-outdeg, kind="stable")
    imb = np.zeros(N, np.int32)          # ke - ko surplus per dst
    pi = np.zeros(N, np.int8)
    qe = np.full((NC, BLOCKS), 64, np.int32)
    qo = np.full((NC, BLOCKS), 64, np.int32)
    qe[:, BLOCKS - 1] = 63  # keep >=1 empty slot of each parity in last block
    qo[:, BLOCKS - 1] = 63
    for n in proc:
        c = node_c[n]
        b = b_of[n]
        ds = d_sorted[sstarts[n]:sstarts[n + 1]]
        if qe[c, b] == 0:
            p = 1
        elif qo[c, b] == 0:
            p = 0
        else:
            p = 1 if imb[ds].sum() > 0 else 0
        pi[n] = p
        if p == 0:
            qe[c, b] -= 1
            imb[ds] += 1
        else:
            qo[c, b] -= 1
            imb[ds] -= 1
    # place nodes at parity-matching positions; core_pos[c, b*128+p] = node or -1
    core_pos = np.full((NC, LSHARD), -1, np.int64)
    for c in range(NC):
        for b in range(BLOCKS):
            lo = b * 128
            hi = min(lo + 128, PER_CORE)
            nodes = core_nodes[c, lo:hi]
            pe = np.nonzero((np.arange(128) + b) % 2 == 0)[0]
            pno = np.nonzero((np.arange(128) + b) % 2 == 1)[0]
            ev = nodes[pi[nodes] == 0]
            od = nodes[pi[nodes] == 1]
            core_pos[c, lo + pe[:len(ev)]] = ev
            core_pos[c, lo + pno[:len(od)]] = od

    posflat = core_pos.ravel()
    valid = posflat >= 0
    node_c[posflat[valid]] = np.repeat(np.arange(NC), LSHARD)[valid]
    node_j[posflat[valid]] = np.tile(np.arange(LSHARD), NC)[valid]
    b_of = node_j // 128
    p_of = node_j % 128
    slot = node_c * LSHARD + p_of * BLOCKS + b_of  # table slot per node

    # dynamic pad slots (empty last-block positions => h' rows are zero)
    emptyc, emptyp = np.nonzero(core_pos[:, (BLOCKS - 1) * 128:] < 0)
    empty_slot = emptyc * LSHARD + emptyp * BLOCKS + (BLOCKS - 1)
    ev_pads = empty_slot[empty_slot % 2 == 0]
    od_pads = empty_slot[empty_slot % 2 == 1]
    assert len(ev_pads) and len(od_pads)
    pair_pad_even = int(ev_pads[0]) // 2
    pair_pad_odd = int(od_pads[0]) // 2

    sslot = slot[src]
    sparity = (sslot & 1).astype(np.int64)
    spair = (sslot >> 1).astype(np.int64)
    assert spair.max() < NPAIR

    # per (dst, parity) counts -> per-block maxima over all cores
    cnt2 = np.bincount(dst * 2 + sparity, minlength=2 * N)
    ke = cnt2[0::2]
    ko = cnt2[1::2]
    KE = np.zeros((NC, LSHARD), np.int64)
    KO = np.zeros((NC, LSHARD), np.int64)
    nflat = posflat[valid]
    KE[node_c[nflat], node_j[nflat]] = ke[nflat]
    KO[node_c[nflat], node_j[nflat]] = ko[nflat]
    Dbe = KE.reshape(NC, BLOCKS, 128).max(axis=(0, 2))
    Dbo = KO.reshape(NC, BLOCKS, 128).max(axis=(0, 2))

    # group blocks: consecutive, <= NBMAX blocks, <= CAPG chunks
    per_block = Dbe + Dbo
    groups = []
    ebase = np.zeros(BLOCKS, np.int64)
    obase = np.zeros(BLOCKS, np.int64)
    chunk_par_list = []
    b = 0
    CHc = 0
    while b < BLOCKS:
        nb = 1
        tot = int(per_block[b])
        while (b + nb < BLOCKS and nb < NBMAX
               and tot + per_block[b + nb] <= CAPG):
            tot += int(per_block[b + nb])
            nb += 1
        gstart = CHc
        eloc, dbe_l, oloc, dbo_l = [], [], [], []
        for k in range(nb):
            eloc.append(CHc - gstart)
            dbe_l.append(int(Dbe[b + k]))
            ebase[b + k] = CHc
            CHc += int(Dbe[b + k])
        ge = CHc - gstart
        for k in range(nb):
            oloc.append(CHc - gstart)
            dbo_l.append(int(Dbo[b + k]))
            obase[b + k] = CHc
            CHc += int(Dbo[b + k])
        go = CHc - gstart - ge
        chunk_par_list.extend([0] * ge + [1] * go)
        groups.append(dict(b0=b, nb=nb, gstart=gstart, ge=ge, go=go,
                           eloc=eloc, dbe=dbe_l, oloc=oloc, dbo=dbo_l))
        b += nb
    CH = CHc
    chunk_par = np.array(chunk_par_list, np.int64)

    # rank of each edge within its (dst, parity) bucket
    sidx = np.lexsort((sparity, dst))
    key_sorted = (dst * 2 + sparity)[sidx]
    first = np.r_[True, key_sorted[1:] != key_sorted[:-1]]
    runid = np.cumsum(first) - 1
    runstart = np.flatnonzero(first)
    rank = np.empty(E, np.int64)
    rank[sidx] = np.arange(E) - runstart[runid]

    dstb = b_of[dst]
    chunk = np.where(sparity == 0, ebase[dstb] + rank, obase[dstb] + rank)

    # fill per-core idx arrays [NC, CH, 128] with pad-pair defaults
    defaults = np.where(chunk_par == 0, pair_pad_even, pair_pad_odd).astype(np.int16)
    arr = np.empty((NC, CH, 128), np.int16)
    arr[:] = defaults[None, :, None]
    arr[node_c[dst], chunk, p_of[dst]] = spair.astype(np.int16)
    # wrap: position i=c*128+p -> row p%16, col c*8 + p//16
    idx16 = arr.reshape(NC, CH, 8, 16).transpose(0, 3, 1, 2).reshape(NC, 16, CH * 8)
    idx128 = np.tile(idx16, (1, 8, 1))

    waste = CH * 128 / (E / NC)
    gkey = tuple((g["b0"], g["nb"], g["gstart"], g["ge"], g["go"],
                  tuple(g["eloc"]), tuple(g["dbe"]),
                  tuple(g["oloc"]), tuple(g["dbo"])) for g in groups)
    return dict(dinv=dinv, core_pos=core_pos, CH=CH, groups=groups,
                gkey=gkey, idx128=idx128, waste=waste)


def make_host_inputs(inputs, P):
    nbf = bf16_np
    x = np.asarray(inputs["x"], np.float32)
    core_pos = P["core_pos"]
    dinv = P["dinv"]
    ident = np.eye(128, dtype=nbf)
    wconv = np.stack([np.asarray(inputs[f"W_conv{i}"], np.float32)
                      for i in range(3)]).astype(nbf)
    bng = np.stack([np.asarray(inputs[f"bn_g{i}"], np.float32)
                    for i in range(3)])[:, :, None]
    bnb = np.stack([np.asarray(inputs[f"bn_b{i}"], np.float32)
                    for i in range(3)])[:, :, None]
    maps = []
    for c in range(NC):
        val = core_pos[c] >= 0
        xT = np.zeros((F_IN, LSHARD), np.float32)
        xT[:, val] = x[core_pos[c][val]].T
        dl = np.zeros((LSHARD,), np.float32)
        dl[val] = dinv[core_pos[c][val]]
        maps.append({
            "xT": xT,
            "idx16": P["idx128"][c],
            "dinvbc": np.broadcast_to(dl, (128, LSHARD)).copy(),
            "W_enc": np.asarray(inputs["W_enc"], np.float32),
            "b_enc": np.asarray(inputs["b_enc"], np.float32)[:, None],
            "W_conv": wconv,
            "bn_g": bng.astype(np.float32),
            "bn_b": bnb.astype(np.float32),
            "W_cls1": np.asarray(inputs["W_cls1"], np.float32).astype(nbf),
            "b_cls1": np.asarray(inputs["b_cls1"], np.float32)[:, None],
            "W_cls2": np.asarray(inputs["W_cls2"], np.float32).astype(nbf),
            "b_cls2": np.asarray(inputs["b_cls2"], np.float32)[:, None],
            "ident": ident,
        })
    return maps


def assemble_output(results, P):
    out = np.zeros((N_REAL, N_CLS), np.float32)
    for c in range(NC):
        val = P["core_pos"][c] >= 0
        out[P["core_pos"][c][val]] = results[c]["outT"][:, val].T
    return out


# ---------------- SPMD runner ----------------
class SpmdRunner:
    def __init__(self, nc, n_cores: int, donate: bool = True):
        install_neuronx_cc_hook()
        self.nc = nc
        self.n_cores = n_cores
        partition_name = nc.partition_id_tensor.name if nc.partition_id_tensor else None

        in_names: list[str] = []
        out_names: list[str] = []
        out_avals = []
        zero_outs: list[np.ndarray] = []
        for alloc in nc.m.functions[0].allocations:
            if not isinstance(alloc, mybir.MemoryLocationSet):
                continue
            name = alloc.memorylocations[0].name
            if alloc.kind == "ExternalInput":
                if name != partition_name:
                    in_names.append(name)
            elif alloc.kind == "ExternalOutput":
                shape = tuple(alloc.tensor_shape)
                dtype = mybir.dt.np(alloc.dtype)
                out_names.append(name)
                out_avals.append(jax.core.ShapedArray(shape, dtype))
                zero_outs.append(np.zeros(shape, dtype))
        self.in_names = in_names
        self.out_names = out_names
        self.out_avals = out_avals
        self.zero_outs = zero_outs
        n_params = len(in_names)
        n_outs = len(out_avals)
        all_names = list(in_names) + list(out_names)
        if partition_name is not None:
            all_names.append(partition_name)

        def _body(*args):
            operands = list(args)
            if partition_name is not None:
                operands.append(partition_id_tensor())
            outs = _bass_exec_p.bind(
                *operands,
                out_avals=tuple(out_avals),
                in_names=tuple(all_names),
                out_names=tuple(out_names),
                lowering_input_output_aliases=(),
                sim_require_finite=True,
                sim_require_nnan=True,
                nc=nc,
            )
            return tuple(outs)

        devices = jax.devices()[:n_cores]
        assert len(devices) == n_cores
        self.mesh = Mesh(np.asarray(devices), ("core",))
        in_specs = (PartitionSpec("core"),) * (n_params + n_outs)
        out_specs = (PartitionSpec("core"),) * n_outs
        donate_argnums = tuple(range(n_params, n_params + n_outs)) if donate else ()
        self.fn = jax.jit(
            shard_map(_body, mesh=self.mesh, in_specs=in_specs,
                      out_specs=out_specs, check_rep=False),
            donate_argnums=donate_argnums,
            keep_unused=True,
        )

    def concat_inputs(self, in_maps):
        n = self.n_cores
        return [
            np.concatenate([np.asarray(in_maps[c][name]) for c in range(n)], axis=0)
            for name in self.in_names
        ]

    def concat_zeros(self):
        return [np.zeros((self.n_cores * z.shape[0], *z.shape[1:]), z.dtype)
                for z in self.zero_outs]

    def run(self, in_maps):
        """Execute once; returns list (per core) of dicts name->np.ndarray."""
        concat_in = self.concat_inputs(in_maps)
        out_arrs = self.fn(*concat_in, *self.concat_zeros())
        res = []
        for c in range(self.n_cores):
            res.append({
                name: np.asarray(out_arrs[i]).reshape(
                    self.n_cores, *self.out_avals[i].shape)[c]
                for i, name in enumerate(self.out_names)
            })
        return res


_CACHE = {}


def kernel(**inputs):
    inputs = {k: np.asarray(v) for k, v in inputs.items()}
    P = preprocess(inputs["edge_index"])
    key = (int(P["CH"]), P["gkey"])
    if key not in _CACHE:
        nc = build(P["groups"], P["CH"])
        _CACHE[key] = SpmdRunner(nc, NC)
    r = _CACHE[key]
    in_maps = make_host_inputs(inputs, P)
    res = r.run(in_maps)
    return assemble_output(res, P)


# revision 17
# speedup vs baseline: 1.1996x; 1.1996x over previous
"""Self-contained 8-core Trainium2 Bass kernel for the BaseGNN problem.

kernel(**inputs) -> np.ndarray [50000, 72] float32.

Strategy (v3): degree-sorted node sharding across 8 NeuronCores (snake-deal
by in-degree, degree-sorted within core so per-128-block max degrees are
tight; slot-table waste is only ~3%). Per conv layer, h' = h*dinv is
transposed block-wise on the TensorEngine (4 blocks per PSUM tile, one
PSUM->SBUF cast each), written to DRAM with a single 1.6MB DMA, and
allgathered into a node-major bf16 table [50176, 128] on every core. Edge
messages are fetched with 128-descriptor GPSIMD indirect DMAs (one per
chunk; int32 row offsets; each dst's chunk list sorted by ascending source
row) into double-buffered per-group SBUF tiles, and aggregated on the
TensorEngine into PSUM (lhsT=gathered chunk, rhs=identity, feature-major
accumulate; groups of <=4 blocks share one 512-col PSUM tile so self-loop
add / W_conv matmul / dinv scale / BN stats run once per group at 512-col
width). BN statistics go through a tiny AllReduce; the encoder and the
two classifier matmuls are fused into the same program.

Measured on trn2 (neuron-profile, core 0): ~4.71 ms, dominated by GPSIMD
SWDGE descriptor generation for the gathers (~8-11 ns per 256B descriptor,
3 x 129k descriptors) - the hard floor for descriptor-per-edge gathers on
this hardware. Relative error vs the fp32 reference: ~7e-3.
"""
import time

import numpy as np
import ml_dtypes

import jax
from jax.sharding import Mesh, PartitionSpec
from jax.experimental.shard_map import shard_map

import concourse.bacc as bacc
import concourse.tile as tile
import concourse.mybir as mybir
from concourse import bass
from concourse.bass2jax import _bass_exec_p, install_neuronx_cc_hook, partition_id_tensor

N = 50000
E = 1000000
bf16_np = ml_dtypes.bfloat16

F_IN = 16
HID = 128
N_CLS = 72
EPS = 1e-5
NC = 8
PER_CORE = 6250
BLOCKS = 49
LSHARD = BLOCKS * 128  # 6272
TOT = NC * LSHARD      # 50176
NPAIR = TOT // 2       # 25088
N_REAL = 50000
CAPG = 110   # max chunks per gather group
NBMAX = 4    # max blocks per group (PSUM tile is 512 cols)

f32 = mybir.dt.float32
bf16 = mybir.dt.bfloat16
i16 = mybir.dt.int16
i32 = mybir.dt.int32


def col_chunks(width=512):
    s = 0
    while s < LSHARD:
        w = min(width, LSHARD - s)
        yield s, w
        s += w


def build(groups, CH):
    """groups: list of dicts with keys b0, nb, gstart, ge, go,
    eloc (per-block local even-chunk base), dbe, oloc, dbo."""
    GW = max(g["gn"] for g in groups)
    nc = bacc.Bacc("TRN2", target_bir_lowering=False, debug=False,
                   enable_asserts=False, num_devices=NC, num_swdge_queues=1)

    # ---- inputs ----
    xT_d = nc.dram_tensor("xT", [F_IN, LSHARD], f32, kind="ExternalInput")
    idx_d = nc.dram_tensor("idx32", [128, CH], i32, kind="ExternalInput")
    dinv_d = nc.dram_tensor("dinvbc", [128, LSHARD], f32, kind="ExternalInput")
    wenc_d = nc.dram_tensor("W_enc", [F_IN, HID], f32, kind="ExternalInput")
    benc_d = nc.dram_tensor("b_enc", [HID, 1], f32, kind="ExternalInput")
    wc_d = nc.dram_tensor("W_conv", [3, HID, HID], bf16, kind="ExternalInput")
    bng_d = nc.dram_tensor("bn_g", [3, HID, 1], f32, kind="ExternalInput")
    bnb_d = nc.dram_tensor("bn_b", [3, HID, 1], f32, kind="ExternalInput")
    wc1_d = nc.dram_tensor("W_cls1", [HID, 64], bf16, kind="ExternalInput")
    bc1_d = nc.dram_tensor("b_cls1", [64, 1], f32, kind="ExternalInput")
    wc2_d = nc.dram_tensor("W_cls2", [64, N_CLS], bf16, kind="ExternalInput")
    bc2_d = nc.dram_tensor("b_cls2", [N_CLS, 1], f32, kind="ExternalInput")
    ident_d = nc.dram_tensor("ident", [128, 128], bf16, kind="ExternalInput")
    out_d = nc.dram_tensor("outT", [N_CLS, LSHARD], f32, kind="ExternalOutput")

    rg = [list(range(NC))]
    NG = len(groups)

    with tile.TileContext(nc) as tc:
        with tc.tile_pool(name="persist", bufs=1) as pp, \
             tc.tile_pool(name="work", bufs=4) as wp, \
             tc.tile_pool(name="psum", bufs=2, space="PSUM") as psp, \
             tc.tile_pool(name="dram", bufs=1, space="DRAM") as dp:

            # ---- persistent SBUF ----
            hT = pp.tile([128, LSHARD], f32, name="hT")
            hpTb = pp.tile([128, LSHARD], bf16, name="hpTb")
            convT = pp.tile([128, LSHARD], f32, name="convT")
            dinv = pp.tile([128, LSHARD], f32, name="dinv")
            bounce_sb = pp.tile([128, LSHARD], bf16, name="bounce_sb")
            idx32 = pp.tile([128, CH], i32, name="idx32")
            identb = pp.tile([128, 128], bf16, name="identb")
            wenc = pp.tile([F_IN, HID], f32, name="wenc")
            benc = pp.tile([HID, 1], f32, name="benc")
            wc = [pp.tile([HID, HID], bf16, name=f"wc{i}") for i in range(3)]
            bng = pp.tile([HID, 3], f32, name="bng")
            bnb = pp.tile([HID, 3], f32, name="bnb")
            wc1 = pp.tile([HID, 64], bf16, name="wc1")
            bc1 = pp.tile([64, 1], f32, name="bc1")
            wc2 = pp.tile([64, N_CLS], bf16, name="wc2")
            bc2 = pp.tile([N_CLS, 1], f32, name="bc2")
            bnst = pp.tile([128, NG * 6], f32, name="bnst")

            nc.sync.dma_start(out=dinv[:], in_=dinv_d.ap())
            nc.sync.dma_start(out=idx32[:], in_=idx_d.ap())
            nc.sync.dma_start(out=identb[:], in_=ident_d.ap())
            nc.sync.dma_start(out=wenc[:], in_=wenc_d.ap())
            nc.sync.dma_start(out=benc[:], in_=benc_d.ap())
            for l in range(3):
                nc.sync.dma_start(out=wc[l][:], in_=wc_d.ap()[l])
                nc.sync.dma_start(out=bng[:, l:l + 1], in_=bng_d.ap()[l])
                nc.sync.dma_start(out=bnb[:, l:l + 1], in_=bnb_d.ap()[l])
            nc.sync.dma_start(out=wc1[:], in_=wc1_d.ap())
            nc.sync.dma_start(out=bc1[:], in_=bc1_d.ap())
            nc.sync.dma_start(out=wc2[:], in_=wc2_d.ap())
            nc.sync.dma_start(out=bc2[:], in_=bc2_d.ap())

            # ---- encoder: hT = relu(Wenc^T @ xT + b), x streamed ----
            for s, w in col_chunks():
                xt = wp.tile([F_IN, 512], f32, tag="xt", bufs=2, name="xt")
                nc.sync.dma_start(out=xt[:, :w], in_=xT_d.ap()[:, s:s + w])
                pse = psp.tile([128, 512], f32, tag="mm", name="pse")
                nc.tensor.matmul(out=pse[:, :w], lhsT=wenc[:], rhs=xt[:, :w],
                                 start=True, stop=True)
                nc.scalar.activation(hT[:, s:s + w], pse[:, :w],
                                     mybir.ActivationFunctionType.Relu,
                                     bias=benc[:, 0:1], scale=1.0)

            # ---- conv layers ----
            for l in range(3):
                # h' = hT * dinv -> bf16
                for s, w in col_chunks():
                    nc.vector.tensor_tensor(out=hpTb[:, s:s + w], in0=hT[:, s:s + w],
                                            in1=dinv[:, s:s + w],
                                            op=mybir.AluOpType.mult)
                # transpose all blocks into bounce_sb (node-major), 4 per PSUM tile
                for t in range((BLOCKS + 3) // 4):
                    b0 = t * 4
                    nb = min(4, BLOCKS - b0)
                    pst = psp.tile([128, 512], f32, tag="mm", name="pst")
                    for k in range(nb):
                        bs = (b0 + k) * 128
                        nc.tensor.matmul(out=pst[:, k * 128:(k + 1) * 128],
                                         lhsT=hpTb[:, bs:bs + 128], rhs=identb[:],
                                         start=True, stop=True)
                    nc.vector.tensor_copy(out=bounce_sb[:, b0 * 128:(b0 + nb) * 128],
                                          in_=pst[:, :nb * 128])
                bounce = dp.tile([128, LSHARD], bf16, name=f"bounce{l}")
                table = dp.tile([TOT, HID], bf16, addr_space="Shared",
                                name=f"table{l}")
                nc.sync.dma_start(out=bounce[:], in_=bounce_sb[:])
                nc.gpsimd.collective_compute(
                    "AllGather", mybir.AluOpType.bypass, replica_groups=rg,
                    ins=[bounce.opt()], outs=[table.opt()])

                # gather groups + aggregation
                for gi, g in enumerate(groups):
                    gn, gstart = g["gn"], g["gstart"]
                    nb, b0 = g["nb"], g["b0"]
                    gb = wp.tile([128, GW * 128], bf16, tag="gb", bufs=2,
                                 name=f"gb{l}_{gi}")
                    for c in range(gn):
                        nc.gpsimd.indirect_dma_start(
                            out=gb[:, c * 128:(c + 1) * 128], out_offset=None,
                            in_=table[:],
                            in_offset=bass.IndirectOffsetOnAxis(
                                ap=idx32[:, gstart + c:gstart + c + 1], axis=0))
                    # aggregate: psa[:, k*128:(k+1)*128] += chunk^T for block's chunks
                    psa = psp.tile([128, 512], f32, tag="agg", name="psa")
                    for k in range(nb):
                        lcs = [g["loc"][k] + i for i in range(g["db"][k])]
                        for i, lc in enumerate(lcs):
                            nc.tensor.matmul(
                                out=psa[:, k * 128:(k + 1) * 128],
                                lhsT=gb[:, lc * 128:(lc + 1) * 128],
                                rhs=identb[:],
                                start=(i == 0), stop=(i == len(lcs) - 1))
                    cs = b0 * 128
                    cw = nb * 128
                    at = wp.tile([128, 512], bf16, tag="at", bufs=2, name="at")
                    nc.vector.tensor_tensor(out=at[:, :cw], in0=psa[:, :cw],
                                            in1=hpTb[:, cs:cs + cw],
                                            op=mybir.AluOpType.add)
                    psc = psp.tile([128, 512], f32, tag="mm", name="psc")
                    nc.tensor.matmul(out=psc[:, :cw], lhsT=wc[l][:], rhs=at[:, :cw],
                                     start=True, stop=True)
                    nc.vector.tensor_tensor(out=convT[:, cs:cs + cw],
                                            in0=psc[:, :cw],
                                            in1=dinv[:, cs:cs + cw],
                                            op=mybir.AluOpType.mult)
                    nc.vector.bn_stats(out=bnst[:, gi * 6:(gi + 1) * 6],
                                       in_=convT[:, cs:cs + cw])

                # global BN stats
                bnagg = wp.tile([128, 2], f32, tag="st", name="bnagg")
                nc.vector.bn_aggr(out=bnagg[:], in_=bnst[:])
                ssum = wp.tile([128, 2], f32, tag="st", name="ssum")
                # ssum[:,0] = LSHARD*mean ; ssum[:,1] = LSHARD*(var+mean^2)
                m2 = wp.tile([128, 1], f32, tag="st1", name="m2")
                nc.vector.tensor_tensor(out=m2[:], in0=bnagg[:, 0:1],
                                        in1=bnagg[:, 0:1], op=mybir.AluOpType.mult)
                nc.vector.tensor_scalar_mul(ssum[:, 0:1], bnagg[:, 0:1],
                                            float(LSHARD))
                q = wp.tile([128, 1], f32, tag="st1", name="q")
                nc.vector.tensor_tensor(out=q[:], in0=bnagg[:, 1:2], in1=m2[:],
                                        op=mybir.AluOpType.add)
                nc.vector.tensor_scalar_mul(ssum[:, 1:2], q[:], float(LSHARD))
                stat_src = dp.tile([128, 2], f32, name=f"stat_src{l}")
                stat_dst = dp.tile([128, 2], f32, addr_space="Shared",
                                   name=f"stat_dst{l}")
                nc.sync.dma_start(out=stat_src[:], in_=ssum[:])
                nc.gpsimd.collective_compute(
                    "AllReduce", mybir.AluOpType.add, replica_groups=rg,
                    ins=[stat_src.opt()], outs=[stat_dst.opt()])
                gstat = wp.tile([128, 2], f32, tag="st", name="gstat")
                nc.sync.dma_start(out=gstat[:], in_=stat_dst[:])
                mu = wp.tile([128, 1], f32, tag="st1", name="mu")
                nc.vector.tensor_scalar_mul(mu[:], gstat[:, 0:1], 1.0 / N_REAL)
                var = wp.tile([128, 1], f32, tag="st1", name="var")
                nc.vector.tensor_scalar_mul(var[:], gstat[:, 1:2], 1.0 / N_REAL)
                mu2 = wp.tile([128, 1], f32, tag="st1", name="mu2")
                nc.vector.tensor_tensor(out=mu2[:], in0=mu[:], in1=mu[:],
                                        op=mybir.AluOpType.mult)
                nc.vector.tensor_tensor(out=var[:], in0=var[:], in1=mu2[:],
                                        op=mybir.AluOpType.subtract)
                nc.vector.tensor_scalar_add(var[:], var[:], EPS)
                rinv = wp.tile([128, 1], f32, tag="st1", name="rinv")
                nc.vector.reciprocal(rinv[:], var[:])
                rs = wp.tile([128, 1], f32, tag="st1", name="rs")
                nc.scalar.sqrt(rs[:], rinv[:])
                gp = wp.tile([128, 1], f32, tag="st1", name="gp")
                nc.vector.tensor_tensor(out=gp[:], in0=bng[:, l:l + 1], in1=rs[:],
                                        op=mybir.AluOpType.mult)
                mgp = wp.tile([128, 1], f32, tag="st1", name="mgp")
                nc.vector.tensor_tensor(out=mgp[:], in0=mu[:], in1=gp[:],
                                        op=mybir.AluOpType.mult)
                bp = wp.tile([128, 1], f32, tag="st1", name="bp")
                nc.vector.tensor_tensor(out=bp[:], in0=bnb[:, l:l + 1], in1=mgp[:],
                                        op=mybir.AluOpType.subtract)

                # bn apply + relu (+ residual)
                for s, w in col_chunks():
                    if l == 0:
                        nc.scalar.activation(hT[:, s:s + w], convT[:, s:s + w],
                                             mybir.ActivationFunctionType.Relu,
                                             bias=bp[:, 0:1], scale=gp[:, 0:1])
                    else:
                        hnew = wp.tile([128, 512], f32, tag="hnew", bufs=2, name="hnew")
                        nc.scalar.activation(hnew[:, :w], convT[:, s:s + w],
                                             mybir.ActivationFunctionType.Relu,
                                             bias=bp[:, 0:1], scale=gp[:, 0:1])
                        nc.vector.tensor_tensor(out=hT[:, s:s + w],
                                                in0=hT[:, s:s + w],
                                                in1=hnew[:, :w],
                                                op=mybir.AluOpType.add)

            # ---- classifier ----
            h4b = pp.tile([64, LSHARD], bf16, name="h4b")
            for s, w in col_chunks():
                nc.vector.tensor_copy(out=hpTb[:, s:s + w], in_=hT[:, s:s + w])
                ps1 = psp.tile([64, 512], f32, tag="mmc", name="ps1")
                nc.tensor.matmul(out=ps1[:, :w], lhsT=wc1[:], rhs=hpTb[:, s:s + w],
                                 start=True, stop=True)
                nc.scalar.activation(h4b[:, s:s + w], ps1[:, :w],
                                     mybir.ActivationFunctionType.Relu,
                                     bias=bc1[:, 0:1], scale=1.0)
            for s, w in col_chunks():
                ps2 = psp.tile([N_CLS, 512], f32, tag="mmc", name="ps2")
                nc.tensor.matmul(out=ps2[:, :w], lhsT=wc2[:], rhs=h4b[:, s:s + w],
                                 start=True, stop=True)
                ot = wp.tile([N_CLS, 512], f32, tag="ot", bufs=2, name="ot")
                nc.vector.tensor_scalar(out=ot[:, :w], in0=ps2[:, :w],
                                        scalar1=bc2[:, 0:1], scalar2=None,
                                        op0=mybir.AluOpType.add)
                nc.sync.dma_start(out=out_d.ap()[:, s:s + w], in_=ot[:, :w])

    nc.compile()
    return nc


# ---------------- host preprocessing ----------------
def preprocess(edge_index):
    src = np.asarray(edge_index[0], dtype=np.int64)
    dst = np.asarray(edge_index[1], dtype=np.int64)
    indeg = np.bincount(dst, minlength=N).astype(np.int64)
    deg = (indeg + 1).astype(np.float32)
    dinv = (1.0 / np.sqrt(deg)).astype(np.float32)

    # snake-deal nodes sorted by indeg desc -> 8 cores x 6250, edge-balanced
    order = np.argsort(-indeg, kind="stable")
    rounds = PER_CORE // 2
    po = order.reshape(rounds, 2, NC)
    core_nodes = np.empty((NC, PER_CORE), dtype=np.int64)
    core_nodes[:, 0::2] = po[:, 0, :].T
    core_nodes[:, 1::2] = po[:, 1, ::-1].T
    for c in range(NC):
        o = np.argsort(-indeg[core_nodes[c]], kind="stable")
        core_nodes[c] = core_nodes[c][o]

    # core_pos[c, j] = node (j = b*128 + p); last block leaves pads at the end
    core_pos = np.full((NC, LSHARD), -1, np.int64)
    core_pos[:, :PER_CORE] = core_nodes
    flat = core_nodes.ravel()
    node_c = np.empty(N, np.int64)
    node_j = np.empty(N, np.int64)
    node_c[flat] = np.repeat(np.arange(NC), PER_CORE)
    node_j[flat] = np.tile(np.arange(PER_CORE), NC)
    b_of = node_j // 128
    p_of = node_j % 128
    slot = node_c * LSHARD + p_of * BLOCKS + b_of  # table row per node
    pad_slot = 0 * LSHARD + 127 * BLOCKS + (BLOCKS - 1)  # core0 j=6271 (pad, h'=0)

    # per-dst degree -> per-block maxima over all cores
    KD = np.zeros((NC, LSHARD), np.int64)
    KD[node_c[flat], node_j[flat]] = indeg[flat]
    Db = KD.reshape(NC, BLOCKS, 128).max(axis=(0, 2))

    # group blocks: consecutive, <= NBMAX blocks, <= CAPG chunks
    groups = []
    cbase = np.zeros(BLOCKS, np.int64)
    b = 0
    CHc = 0
    while b < BLOCKS:
        nb = 1
        tot = int(Db[b])
        while (b + nb < BLOCKS and nb < NBMAX
               and tot + Db[b + nb] <= CAPG):
            tot += int(Db[b + nb])
            nb += 1
        gstart = CHc
        loc, db_l = [], []
        for k in range(nb):
            loc.append(CHc - gstart)
            db_l.append(int(Db[b + k]))
            cbase[b + k] = CHc
            CHc += int(Db[b + k])
        groups.append(dict(b0=b, nb=nb, gstart=gstart, gn=CHc - gstart,
                           loc=loc, db=db_l))
        b += nb
    CH = CHc

    # rank of each edge within its dst, ordered by ascending source row
    # (consecutive gather descs then read roughly increasing addresses)
    sidx = np.lexsort((slot[src], dst))
    key_sorted = dst[sidx]
    first = np.r_[True, key_sorted[1:] != key_sorted[:-1]]
    runid = np.cumsum(first) - 1
    runstart = np.flatnonzero(first)
    rank = np.empty(E, np.int64)
    rank[sidx] = np.arange(E) - runstart[runid]

    chunk = cbase[b_of[dst]] + rank

    # per-core idx arrays [NC, 128, CH] int32 with pad-row defaults
    arr = np.full((NC, CH, 128), pad_slot, np.int32)
    arr[node_c[dst], chunk, p_of[dst]] = slot[src].astype(np.int32)
    idx32 = np.ascontiguousarray(arr.transpose(0, 2, 1))  # [NC, 128, CH]

    waste = CH * 128 / (E / NC)
    gkey = tuple((g["b0"], g["nb"], g["gstart"], g["gn"],
                  tuple(g["loc"]), tuple(g["db"])) for g in groups)
    return dict(dinv=dinv, core_pos=core_pos, CH=CH, groups=groups,
                gkey=gkey, idx32=idx32, waste=waste)


def make_host_inputs(inputs, P):
    nbf = bf16_np
    x = np.asarray(inputs["x"], np.float32)
    core_pos = P["core_pos"]
    dinv = P["dinv"]
    ident = np.eye(128, dtype=nbf)
    wconv = np.stack([np.asarray(inputs[f"W_conv{i}"], np.float32)
                      for i in range(3)]).astype(nbf)
    bng = np.stack([np.asarray(inputs[f"bn_g{i}"], np.float32)
                    for i in range(3)])[:, :, None]
    bnb = np.stack([np.asarray(inputs[f"bn_b{i}"], np.float32)
                    for i in range(3)])[:, :, None]
    maps = []
    for c in range(NC):
        val = core_pos[c] >= 0
        xT = np.zeros((F_IN, LSHARD), np.float32)
        xT[:, val] = x[core_pos[c][val]].T
        dl = np.zeros((LSHARD,), np.float32)
        dl[val] = dinv[core_pos[c][val]]
        maps.append({
            "xT": xT,
            "idx32": P["idx32"][c],
            "dinvbc": np.broadcast_to(dl, (128, LSHARD)).copy(),
            "W_enc": np.asarray(inputs["W_enc"], np.float32),
            "b_enc": np.asarray(inputs["b_enc"], np.float32)[:, None],
            "W_conv": wconv,
            "bn_g": bng.astype(np.float32),
            "bn_b": bnb.astype(np.float32),
            "W_cls1": np.asarray(inputs["W_cls1"], np.float32).astype(nbf),
            "b_cls1": np.asarray(inputs["b_cls1"], np.float32)[:, None],
            "W_cls2": np.asarray(inputs["W_cls2"], np.float32).astype(nbf),
            "b_cls2": np.asarray(inputs["b_cls2"], np.float32)[:, None],
            "ident": ident,
        })
    return maps


def assemble_output(results, P):
    out = np.zeros((N_REAL, N_CLS), np.float32)
    for c in range(NC):
        val = P["core_pos"][c] >= 0
        out[P["core_pos"][c][val]] = results[c]["outT"][:, val].T
    return out


# ---------------- SPMD runner ----------------
class SpmdRunner:
    def __init__(self, nc, n_cores: int, donate: bool = True):
        install_neuronx_cc_hook()
        self.nc = nc
        self.n_cores = n_cores
        partition_name = nc.partition_id_tensor.name if nc.partition_id_tensor else None

        in_names: list[str] = []
        out_names: list[str] = []
        out_avals = []
        zero_outs: list[np.ndarray] = []
        for alloc in nc.m.functions[0].allocations:
            if not isinstance(alloc, mybir.MemoryLocationSet):
                continue
            name = alloc.memorylocations[0].name
            if alloc.kind == "ExternalInput":
                if name != partition_name:
                    in_names.append(name)
            elif alloc.kind == "ExternalOutput":
                shape = tuple(alloc.tensor_shape)
                dtype = mybir.dt.np(alloc.dtype)
                out_names.append(name)
                out_avals.append(jax.core.ShapedArray(shape, dtype))
                zero_outs.append(np.zeros(shape, dtype))
        self.in_names = in_names
        self.out_names = out_names
        self.out_avals = out_avals
        self.zero_outs = zero_outs
        n_params = len(in_names)
        n_outs = len(out_avals)
        all_names = list(in_names) + list(out_names)
        if partition_name is not None:
            all_names.append(partition_name)

        def _body(*args):
            operands = list(args)
            if partition_name is not None:
                operands.append(partition_id_tensor())
            outs = _bass_exec_p.bind(
                *operands,
                out_avals=tuple(out_avals),
                in_names=tuple(all_names),
                out_names=tuple(out_names),
                lowering_input_output_aliases=(),
                sim_require_finite=True,
                sim_require_nnan=True,
                nc=nc,
            )
            return tuple(outs)

        devices = jax.devices()[:n_cores]
        assert len(devices) == n_cores
        self.mesh = Mesh(np.asarray(devices), ("core",))
        in_specs = (PartitionSpec("core"),) * (n_params + n_outs)
        out_specs = (PartitionSpec("core"),) * n_outs
        donate_argnums = tuple(range(n_params, n_params + n_outs)) if donate else ()
        self.fn = jax.jit(
            shard_map(_body, mesh=self.mesh, in_specs=in_specs,
                      out_specs=out_specs, check_rep=False),
            donate_argnums=donate_argnums,
            keep_unused=True,
        )

    def concat_inputs(self, in_maps):
        n = self.n_cores
        return [
            np.concatenate([np.asarray(in_maps[c][name]) for c in range(n)], axis=0)
            for name in self.in_names
        ]

    def concat_zeros(self):
        return [np.zeros((self.n_cores * z.shape[0], *z.shape[1:]), z.dtype)
                for z in self.zero_outs]

    def run(self, in_maps):
        """Execute once; returns list (per core) of dicts name->np.ndarray."""
        concat_in = self.concat_inputs(in_maps)
        out_arrs = self.fn(*concat_in, *self.concat_zeros())
        res = []
        for c in range(self.n_cores):
            res.append({
                name: np.asarray(out_arrs[i]).reshape(
                    self.n_cores, *self.out_avals[i].shape)[c]
                for i, name in enumerate(self.out_names)
            })
        return res


_CACHE = {}


def kernel(**inputs):
    inputs = {k: np.asarray(v) for k, v in inputs.items()}
    P = preprocess(inputs["edge_index"])
    key = (int(P["CH"]), P["gkey"])
    if key not in _CACHE:
        nc = build(P["groups"], P["CH"])
        _CACHE[key] = SpmdRunner(nc, NC)
    r = _CACHE[key]
    in_maps = make_host_inputs(inputs, P)
    res = r.run(in_maps)
    return assemble_output(res, P)
